# revision 8
# baseline (speedup 1.0000x reference)
"""CARAFE upsample (N=4, C=256, 64x64 -> 128x128, K=5, SF=2) on 8 NeuronCores.

Bass/Tile kernel, SPMD over 8 cores: core k handles batch n = k//2 and
row-half hh = k%2 (32 source rows, full 256 channels).

Per-core pipeline:
  1. load x row-window [256, 36, 64] fp32 (host zero-padded rows), cast bf16
  2. PE-transpose x -> xT [66 p (x col w, +2 zero rows), 36 r, 2 ct, 128 c]
  3. compressor 1x1 conv (PE): comp [64, 34 r, 66 wpad] bf16
  4. per 8-row chunk: encoder 3x3 conv (9 shifted matmuls, output channels
     at partitions 32*(2i+j) + 5*dx + dy) -> softmax over the 25 taps
     (exp on ACT, tap-sum + reciprocal + broadcast via tiny matmuls)
     -> mask2 [25 p (k'=5dx+dy), 32 h, 64 w, 4 ij] bf16
  5. banded scatter mask2 -> DRAM scratch Bd (diagonal strides legal on the
     flat DRAM side; 2 guard rows at the front absorb w+dx-2 < 0), then
     reload per h as B [66 p, 5 dy, 256 (w,ij)]
  6. reassembly per (h, ct): 5 PSUM-accumulating matmuls
       psum[c, (w,i,j)] = sum_dy xT_row(h+dy)^T @ B(h, dy)
  7. ACT evac reorders (w,i,j)->(i,w,j), DMA out rows 2h, 2h+1.

Contract: kernel(**inputs) -> full (4, 256, 128, 128) float32.
"""
import json
import numpy as np
import ml_dtypes

import concourse.bass as bass
import concourse.mybir as mybir
from concourse.tile import TileContext
from concourse import bass_utils

BF16 = ml_dtypes.bfloat16

H = 64
W = 64
C = 256
CC = 64
NCORES = 8
HC = 32           # rows per core
RW = 36           # loaded row window (HC + 2*2 halo)
PB = 5 * 256      # banded row elems per h (5 dy blocks of (w,ij)=256)
SH = 68 * PB      # Bd elems per h: 2 guard rows + 66 data rows


# ---------------------------------------------------------------------------
# BIR legalization: this walrus build accepts at most one sync-wait per
# instruction; hoist extras into standalone EventSemaphore instructions.
def _legalize_bir_json(bir: bytes) -> bytes:
    m = json.loads(bir)
    for fn in m.get("functions", []):
        for blk in fn.get("blocks", []):
            out = []
            for inst in blk.get("instructions", []):
                si = inst.get("sync_info") or {}
                waits = si.get("on_wait") or []
                if len(waits) > 1:
                    for k, wcond in enumerate(waits[:-1]):
                        out.append({
                            "debug": inst.get("debug", 0),
                            "engine": inst.get("engine"),
                            "ins": [],
                            "name": f"{inst.get('name', 'I')}_hw{k}",
                            "opcode": "EventSemaphore",
                            "outs": [],
                            "sync_info": {"on_update": [], "on_wait": [wcond]},
                        })
                    si = dict(si)
                    si["on_wait"] = [waits[-1]]
                    inst = dict(inst)
                    inst["sync_info"] = si
                out.append(inst)
            blk["instructions"] = out
    return json.dumps(m).encode()


_patched = False


def _install_legalizer():
    global _patched
    if _patched:
        return
    _patched = True
    orig = bass_utils.compile_bir_kernel

    def patched(bir_json, tmpdir, neff_name="file.neff"):
        if isinstance(bir_json, str):
            bir_json = bir_json.encode()
        return orig(_legalize_bir_json(bir_json), tmpdir, neff_name)

    bass_utils.compile_bir_kernel = patched
    try:
        from concourse import bass2jax
        bass2jax.compile_bir_kernel = patched
    except Exception:
        pass


# ---------------------------------------------------------------------------
def build_carafe(nc: bass.Bass):
    fp32 = mybir.dt.float32
    bf16 = mybir.dt.bfloat16
    Copy = mybir.ActivationFunctionType.Copy
    Ident = mybir.ActivationFunctionType.Identity
    Exp = mybir.ActivationFunctionType.Exp

    xs = nc.dram_tensor("xs", (C, RW, W), fp32, kind="ExternalInput")
    wc = nc.dram_tensor("wc", (2, 128, CC), bf16, kind="ExternalInput")
    we = nc.dram_tensor("we", (CC, 9, 128), bf16, kind="ExternalInput")
    bc_d = nc.dram_tensor("bc", (CC, 1), fp32, kind="ExternalInput")
    be_d = nc.dram_tensor("be", (128, 1), fp32, kind="ExternalInput")
    sel_d = nc.dram_tensor("sel", (128, 4), bf16, kind="ExternalInput")
    selb_d = nc.dram_tensor("selb", (4, 128), bf16, kind="ExternalInput")
    id_d = nc.dram_tensor("ident", (128, 128), bf16, kind="ExternalInput")
    outp = nc.dram_tensor("out", (C, 2 * HC, 2 * W), fp32, kind="ExternalOutput")
    Bd = nc.dram_tensor("Bd", (HC * SH,), bf16, kind="Internal")

    with nc.allow_low_precision(reason="bf16 pipeline, tol 2e-2"), \
         TileContext(nc) as tc:
        with (
            tc.tile_pool(name="const", bufs=1) as constp,
            tc.tile_pool(name="data", bufs=1) as datap,
            tc.tile_pool(name="bsb", bufs=4) as bsbp,
            tc.tile_pool(name="stagep", bufs=4) as stagep,
            tc.tile_pool(name="pst", bufs=2, space="PSUM") as pst,
            tc.tile_pool(name="psc", bufs=2, space="PSUM") as psc,
            tc.tile_pool(name="pss", bufs=2, space="PSUM") as pss,
            tc.tile_pool(name="psr", bufs=2, space="PSUM") as psr,
        ):
            wc_t = constp.tile([128, 2, CC], bf16, tag="wc")
            we_t = constp.tile([CC, 9, 128], bf16, tag="we")
            bc_t = constp.tile([CC, 1], fp32, tag="bc")
            be_t = constp.tile([128, 1], fp32, tag="be")
            sel_t = constp.tile([128, 4], bf16, tag="sel")
            selb_t = constp.tile([4, 128], bf16, tag="selb")
            id_t = constp.tile([128, 128], bf16, tag="ident")
            zero_t = constp.tile([68, PB], bf16, tag="zero")

            nc.sync.dma_start(wc_t[:, :, :], wc.rearrange("t p c -> p t c"))
            nc.sync.dma_start(we_t[:, :, :], we[:, :, :])
            nc.sync.dma_start(bc_t[:, :], bc_d[:, :])
            nc.sync.dma_start(be_t[:, :], be_d[:, :])
            nc.sync.dma_start(sel_t[:, :], sel_d[:, :])
            nc.sync.dma_start(selb_t[:, :], selb_d[:, :])
            nc.sync.dma_start(id_t[:, :], id_d[:, :])
            nc.vector.memset(zero_t[:, :], 0.0)

            # zero-fill the banded scratch (off-band entries must be 0)
            for h in range(HC):
                nc.sync.dma_start(
                    Bd[h * SH:(h + 1) * SH].rearrange("(p f) -> p f", p=68),
                    zero_t[:, :])

            xsb = datap.tile([128, 2, RW, W], fp32, tag="xs")
            nc.sync.dma_start(
                xsb[:, :, :, :],
                xs.rearrange("(t p) r w -> p t r w", p=128))
            xb = datap.tile([128, 2, RW, W], bf16, tag="xb")
            nc.vector.tensor_copy(xb[:, :, :, :], xsb[:, :, :, :])

            # --- transpose x into xT [66 p = x col, (r, ct, c)] ------------
            xT = datap.tile([66, RW, 2, 128], bf16, tag="xT")
            nc.vector.memset(xT[64:66, :, :, :], 0.0)
            for ct in range(2):
                for r0 in range(0, RW, 4):
                    ps = pst.tile([64, 512], bf16, tag="pst")
                    for i in range(4):
                        nc.tensor.transpose(
                            ps[:, i * 128:(i + 1) * 128],
                            xb[:, ct, r0 + i, :],
                            id_t[:, :])
                    nc.vector.tensor_copy(
                        xT[0:64, r0:r0 + 4, ct, :],
                        ps[:, :].rearrange("p (i c) -> p i c", i=4))

            # --- compressor 1x1 conv --------------------------------------
            # comp rows rc (0..34) = xs rows rc+1;  w padded to 66 cols
            comp = datap.tile([CC, 34, 66], bf16, tag="comp")
            nc.vector.memset(comp[:, :, 0], 0.0)
            nc.vector.memset(comp[:, :, 65], 0.0)
            for r0 in range(1, 35, 8):
                nr = min(8, 35 - r0)
                ps = psc.tile([128, 512], fp32, tag="psc")
                for ct in range(2):
                    nc.tensor.matmul(
                        ps[0:CC, 0:nr * W],
                        wc_t[:, ct, :],
                        xb[:, ct, r0:r0 + nr, :],
                        start=(ct == 0), stop=(ct == 1))
                nc.scalar.activation(
                    comp[:, r0 - 1:r0 - 1 + nr, 1:65],
                    ps[0:CC, 0:nr * W].rearrange("p (r w) -> p r w", r=nr),
                    Ident, bias=bc_t[:, :], scale=1.0)

            expT = datap.tile([128, HC * W], bf16, tag="expT")
            recip_sb = datap.tile([4, HC * W], bf16, tag="recip")
            recipb = datap.tile([128, HC * W], bf16, tag="recipb")
            mask2 = datap.tile([25, HC, W, 4], bf16, tag="mask2")

            for hc in range(4):
                h0 = hc * 8
                fsl = slice(h0 * W, (h0 + 8) * W)
                # --- encoder 3x3 conv (9 shifted matmuls) -----------------
                pse = psc.tile([128, 512], fp32, tag="psc")
                t = 0
                for ky in range(3):
                    for kx in range(3):
                        nc.tensor.matmul(
                            pse[:, :],
                            we_t[:, t, :],
                            comp[:, h0 + ky:h0 + ky + 8, kx:kx + 64],
                            start=(t == 0), stop=(t == 8))
                        t += 1
                nc.scalar.activation(expT[:, fsl], pse[:, :], Exp,
                                     bias=be_t[:, :], scale=1.0)
                # --- softmax denominator / reciprocal ---------------------
                psd = pss.tile([128, 512], fp32, tag="pss")
                nc.tensor.matmul(psd[0:4, :], sel_t[:, :], expT[:, fsl],
                                 start=True, stop=True)
                nc.vector.reciprocal(recip_sb[:, fsl], psd[0:4, :])
                psb = pss.tile([128, 512], fp32, tag="pss")
                nc.tensor.matmul(psb[:, :], selb_t[:, :], recip_sb[:, fsl],
                                 start=True, stop=True)
                nc.scalar.activation(recipb[:, fsl], psb[:, :], Copy)
                # --- normalize into mask2 ---------------------------------
                for ij in range(4):
                    nc.vector.tensor_mul(
                        mask2[:, h0:h0 + 8, :, ij],
                        expT[32 * ij:32 * ij + 25, fsl].rearrange(
                            "p (h w) -> p h w", h=8),
                        recipb[32 * ij:32 * ij + 25, fsl].rearrange(
                            "p (h w) -> p h w", h=8))

                # --- banded scatter + reassembly per row ------------------
                for hh in range(8):
                    h = h0 + hh
                    src = bass.AP(mask2[:, :, :, :].tensor, h * 256,
                                  [[HC * W * 4, 25], [4, W], [1, 4]])
                    dst = bass.AP(Bd[:].tensor, h * SH,
                                  [[256, 25], [PB + 4, W], [1, 4]])
                    nc.sync.dma_start(dst, src)
                    bsb = bsbp.tile([66, 5, 256], bf16, tag="bsb")
                    nc.sync.dma_start(
                        bsb[:, :, :],
                        Bd[h * SH + 2 * PB:(h + 1) * SH].rearrange(
                            "(p d f) -> p d f", p=66, d=5))
                    for ct in range(2):
                        psm = psr.tile([128, 64, 2, 2], fp32, tag="psr")
                        for dy in range(5):
                            nc.tensor.matmul(
                                psm[:, :, :, :],
                                xT[:, h + dy, ct, :],
                                bsb[:, dy, :],
                                start=(dy == 0), stop=(dy == 4))
                        stage = stagep.tile([128, 2, 64, 2], fp32, tag="stage")
                        nc.scalar.activation(
                            stage[:, :, :, :],
                            psm[:, :, :, :].rearrange("p w i j -> p i w j"),
                            Copy)
                        nc.sync.dma_start(
                            outp[ct * 128:(ct + 1) * 128,
                                 2 * h:2 * h + 2, :],
                            stage[:, :, :, :].rearrange("p i w j -> p i (w j)"))
    return nc


# ---------------------------------------------------------------------------
_cache = {}


def _get_nc():
    if "nc" not in _cache:
        _install_legalizer()
        nc = bass.Bass()
        build_carafe(nc)
        _cache["nc"] = nc
    return _cache["nc"]


def _prep_inputs(x, Wc, bc, We, be):
    x = np.asarray(x, np.float32)
    N = x.shape[0]
    WcT = np.ascontiguousarray(
        np.transpose(np.asarray(Wc)[:, :, 0, 0], (1, 0)).reshape(2, 128, CC)
    ).astype(BF16)
    # Encoder channel layout: partition 32*(2i+j) + 5*dx + dy
    # orig channel = (5*dy+dx)*4 + 2*i+j ; unused partitions zero.
    We = np.asarray(We)
    be = np.asarray(be)
    Wep = np.zeros((128, CC, 3, 3), We.dtype)
    bep = np.zeros((128, 1), np.float32)
    sel = np.zeros((128, 4), BF16)
    selb = np.zeros((4, 128), BF16)
    for ij in range(4):
        for dx in range(5):
            for dy in range(5):
                p = 32 * ij + 5 * dx + dy
                o = (5 * dy + dx) * 4 + ij
                Wep[p] = We[o]
                bep[p, 0] = be[o]
                sel[p, ij] = 1
                selb[ij, p] = 1
    wet = np.ascontiguousarray(
        Wep.transpose(1, 2, 3, 0).reshape(CC, 9, 128)).astype(BF16)
    ident = np.eye(128, dtype=BF16)
    bcx = np.asarray(bc).astype(np.float32).reshape(CC, 1)

    xp = np.zeros((N, C, H + 4, W), np.float32)
    xp[:, :, 2:H + 2] = x
    in_maps = []
    for core in range(NCORES):
        n, hh = core // 2, core % 2
        xs = np.ascontiguousarray(xp[n, :, hh * HC:hh * HC + RW, :])
        in_maps.append({
            "xs": xs, "wc": WcT, "we": wet, "bc": bcx, "be": bep,
            "sel": sel, "selb": selb, "ident": ident,
        })
    return in_maps


def kernel(x, Wc, bc, We, be):
    nc = _get_nc()
    in_maps = _prep_inputs(x, Wc, bc, We, be)
    res = bass_utils.run_bass_kernel_spmd(nc, in_maps, list(range(NCORES)))
    N = np.asarray(x).shape[0]
    full = np.empty((N, C, 2 * H, 2 * W), np.float32)
    for core in range(NCORES):
        n, hh = core // 2, core % 2
        full[n, :, 64 * hh:64 * hh + 64, :] = res.results[core]["out"]
    return full


# revision 40
# speedup vs baseline: 2.7623x; 2.7623x over previous
"""CARAFE upsample (N=4, C=256, 64x64 -> 128x128, K=5, SF=2) on 8 NeuronCores.

Bass/Tile kernel, SPMD over 8 cores: core k handles batch n = k//2 and
row-half hh = k%2 (32 source rows, full 256 channels).

Per-core pipeline:
  1. load x row-window [256, 36, 64] fp32 (host zero-padded rows), cast bf16
  2. PE-transpose x -> xT [66 p (x col w, +2 zero rows), 36 r, 2 ct, 128 c]
  3. compressor 1x1 conv (PE): comp [64, 34 r, 66 wpad] bf16
  4. per 8-row chunk: encoder 3x3 conv (9 shifted matmuls, output channels
     at partitions 32*(2i+j) + 5*dx + dy) -> softmax over the 25 taps
     (exp on ACT, tap-sum + reciprocal + broadcast via tiny matmuls)
     -> mask2 [25 p (k'=5dx+dy), 32 h, 64 w, 4 ij] bf16
  5. banded scatter mask2 -> DRAM scratch Bd (diagonal strides legal on the
     flat DRAM side; 2 guard rows at the front absorb w+dx-2 < 0), then
     reload per h as B [66 p, 5 dy, 256 (w,ij)]
  6. reassembly per (h, ct): 5 PSUM-accumulating matmuls
       psum[c, (w,i,j)] = sum_dy xT_row(h+dy)^T @ B(h, dy)
  7. ACT evac reorders (w,i,j)->(i,w,j), DMA out rows 2h, 2h+1.

Contract: kernel(**inputs) -> full (4, 256, 128, 128) float32.
"""
import json
import numpy as np
import ml_dtypes

import concourse.bass as bass
import concourse.mybir as mybir
from concourse.tile import TileContext
from concourse import bass_utils

BF16 = ml_dtypes.bfloat16

H = 64
W = 64
C = 256
CC = 64
NCORES = 8
HC = 32           # rows per core
RW = 36           # loaded row window (HC + 2*2 halo)
PB = 5 * 512      # banded row elems per row-PAIR (5 dy blocks of (w,ij,e)=512)
SH = 68 * PB      # Bd elems per pair: 2 guard rows + 66 data rows


# ---------------------------------------------------------------------------
# BIR legalization: this walrus build accepts at most one sync-wait per
# instruction; hoist extras into standalone EventSemaphore instructions.
def _legalize_bir_json(bir: bytes) -> bytes:
    m = json.loads(bir)
    for fn in m.get("functions", []):
        for blk in fn.get("blocks", []):
            out = []
            for inst in blk.get("instructions", []):
                si = inst.get("sync_info") or {}
                waits = si.get("on_wait") or []
                if len(waits) > 1:
                    for k, wcond in enumerate(waits[:-1]):
                        out.append({
                            "debug": inst.get("debug", 0),
                            "engine": inst.get("engine"),
                            "ins": [],
                            "name": f"{inst.get('name', 'I')}_hw{k}",
                            "opcode": "EventSemaphore",
                            "outs": [],
                            "sync_info": {"on_update": [], "on_wait": [wcond]},
                        })
                    si = dict(si)
                    si["on_wait"] = [waits[-1]]
                    inst = dict(inst)
                    inst["sync_info"] = si
                out.append(inst)
            blk["instructions"] = out
    return json.dumps(m).encode()


_patched = False


def _install_legalizer():
    global _patched
    if _patched:
        return
    _patched = True
    orig = bass_utils.compile_bir_kernel

    def patched(bir_json, tmpdir, neff_name="file.neff"):
        if isinstance(bir_json, str):
            bir_json = bir_json.encode()
        return orig(_legalize_bir_json(bir_json), tmpdir, neff_name)

    bass_utils.compile_bir_kernel = patched
    try:
        from concourse import bass2jax
        bass2jax.compile_bir_kernel = patched
    except Exception:
        pass


# ---------------------------------------------------------------------------
def build_carafe(nc: bass.Bass):
    fp32 = mybir.dt.float32
    bf16 = mybir.dt.bfloat16
    Copy = mybir.ActivationFunctionType.Copy
    Ident = mybir.ActivationFunctionType.Identity
    Exp = mybir.ActivationFunctionType.Exp

    xs = nc.dram_tensor("xs", (C, RW, W), bf16, kind="ExternalInput")
    wc = nc.dram_tensor("wc", (2, 128, CC), bf16, kind="ExternalInput")
    we = nc.dram_tensor("we", (CC, 9, 128), bf16, kind="ExternalInput")
    bc_d = nc.dram_tensor("bc", (CC, 1), fp32, kind="ExternalInput")
    be_d = nc.dram_tensor("be", (128, 1), fp32, kind="ExternalInput")
    sel_d = nc.dram_tensor("sel", (128, 4), bf16, kind="ExternalInput")
    selb_d = nc.dram_tensor("selb", (4, 128), bf16, kind="ExternalInput")
    id_d = nc.dram_tensor("ident", (128, 128), bf16, kind="ExternalInput")
    outp = nc.dram_tensor("out", (C, 2 * HC, 2 * W), bf16, kind="ExternalOutput")
    Bd = nc.dram_tensor("Bd", (HC // 2 * SH,), bf16, kind="Internal")

    with nc.allow_low_precision(reason="bf16 pipeline, tol 2e-2"), \
         TileContext(nc) as tc:
        with (
            tc.tile_pool(name="const", bufs=1) as constp,
            tc.tile_pool(name="data", bufs=1) as datap,
            tc.tile_pool(name="stagep", bufs=6) as stagep,
            tc.tile_pool(name="pst", bufs=2, space="PSUM") as pst,
            tc.tile_pool(name="psc", bufs=2, space="PSUM") as psc,
            tc.tile_pool(name="pss", bufs=1, space="PSUM") as pss,
            tc.tile_pool(name="psr", bufs=3, space="PSUM") as psr,
        ):
            wc_t = constp.tile([128, 2, CC], bf16, tag="wc")
            we_t = constp.tile([CC, 9, 128], bf16, tag="we")
            bc_t = constp.tile([CC, 1], fp32, tag="bc")
            be_t = constp.tile([128, 1], fp32, tag="be")
            sel_t = constp.tile([128, 4], bf16, tag="sel")
            selb_t = constp.tile([4, 128], bf16, tag="selb")
            id_t = constp.tile([128, 128], bf16, tag="ident")
            zero_t = constp.tile([68, PB], bf16, tag="zero")

            nc.sync.dma_start(id_t[:, :], id_d[:, :])
            xb = datap.tile([128, 2, RW, W], bf16, tag="xb")
            xsr = xs.rearrange("(t p) r w -> p t r w", p=128)
            for ct in range(2):
                for r0 in range(0, RW, 12):
                    nc.sync.dma_start(xb[:, ct, r0:r0 + 12, :],
                                      xsr[:, ct, r0:r0 + 12, :])
            nc.sync.dma_start(wc_t[:, :, :], wc.rearrange("t p c -> p t c"))
            nc.sync.dma_start(we_t[:, :, :], we[:, :, :])
            nc.sync.dma_start(bc_t[:, :], bc_d[:, :])
            nc.sync.dma_start(be_t[:, :], be_d[:, :])
            nc.sync.dma_start(sel_t[:, :], sel_d[:, :])
            nc.sync.dma_start(selb_t[:, :], selb_d[:, :])
            nc.vector.memset(zero_t[:, :], 0.0)
            # zero-fill all banded scratch rows up front (no deps)
            for hp in range(HC // 2):
                nc.gpsimd.dma_start(
                    Bd[hp * SH:(hp + 1) * SH].rearrange("(p f) -> p f", p=68),
                    zero_t[:, 0:PB])

            # --- transpose x into xT [66 p = x col, (r, ct, c)] ------------
            xT = datap.tile([66, RW, 2, 128], bf16, tag="xT")
            for q4 in range(4):
                nc.sync.dma_start(
                    xT[64:66, 9 * q4:9 * q4 + 9, :, :].rearrange(
                        "p r t c -> p (r t c)"),
                    zero_t[0:2, 0:9 * 2 * 128])
            for ct in range(2):
                for r0 in range(0, RW, 4):
                    ps = pst.tile([64, 512], bf16, tag="pst")
                    for i in range(4):
                        nc.tensor.transpose(
                            ps[:, i * 128:(i + 1) * 128],
                            xb[:, ct, r0 + i, :],
                            id_t[:, :])
                    nc.vector.tensor_copy(
                        xT[0:64, r0:r0 + 4, ct, :],
                        ps[:, :].rearrange("p (i c) -> p i c", i=4))

            # --- compressor 1x1 conv --------------------------------------
            # comp rows rc (0..34) = xs rows rc+1;  w padded to 66 cols
            comp = datap.tile([CC, 34, 66], bf16, tag="comp")
            nc.vector.memset(comp[:, :, 0], 0.0)
            nc.vector.memset(comp[:, :, 65], 0.0)
            for r0 in range(1, 35, 8):
                nr = min(8, 35 - r0)
                ps = psc.tile([128, 512], fp32, tag="psc")
                for ct in range(2):
                    nc.tensor.matmul(
                        ps[0:CC, 0:nr * W],
                        wc_t[:, ct, :],
                        xb[:, ct, r0:r0 + nr, :],
                        start=(ct == 0), stop=(ct == 1))
                nc.scalar.activation(
                    comp[:, r0 - 1:r0 - 1 + nr, 1:65],
                    ps[0:CC, 0:nr * W].rearrange("p (r w) -> p r w", r=nr),
                    Ident, bias=bc_t[:, :], scale=1.0)

            # persistent row-pair band tiles [66 p, s=7 slots, (w,ij,e)=512];
            # slots 0 and 6 are permanent zeros (band edge padding).
            pair_t = [datap.tile([66, 7, 512], bf16, tag=f"pair{i}",
                                 name=f"pair{i}")
                      for i in range(4)]
            for i in range(4):
                nc.sync.dma_start(pair_t[i][:, 0, :], zero_t[0:66, 0:512])
                nc.sync.dma_start(pair_t[i][:, 6, :], zero_t[0:66, 0:512])

            expT = datap.tile([128, HC * W], bf16, tag="expT")
            recip_sb = datap.tile([4, HC * W], bf16, tag="recip")
            recipb = datap.tile([128, HC * W], bf16, tag="recipb")
            # mask2 element layout: (pair hp, w, ij, e); slot e=1 holds the
            # even row 2hp, e=0 the odd row 2hp+1 (so the reassembly moving
            # operand has all-positive strides).
            mask2 = datap.tile([25, HC // 2, W, 4, 2], bf16, tag="mask2")

            def mask_chunk(hc):
                h0 = hc * 8
                fsl = slice(h0 * W, (h0 + 8) * W)
                # --- encoder 3x3 conv (9 shifted matmuls) -----------------
                pse = psc.tile([128, 512], fp32, tag="psc")
                t = 0
                for ky in range(3):
                    for kx in range(3):
                        nc.tensor.matmul(
                            pse[:, :],
                            we_t[:, t, :],
                            comp[:, h0 + ky:h0 + ky + 8, kx:kx + 64],
                            start=(t == 0), stop=(t == 8))
                        t += 1
                nc.scalar.activation(expT[:, fsl], pse[:, :], Exp,
                                     bias=be_t[:, :], scale=1.0)
                # --- softmax denominator / reciprocal ---------------------
                psd = pss.tile([128, 512], fp32, tag="pss")
                nc.tensor.matmul(psd[0:4, :], sel_t[:, :], expT[:, fsl],
                                 start=True, stop=True)
                nc.vector.reciprocal(recip_sb[:, fsl], psd[0:4, :])
                psb = pss.tile([128, 512], fp32, tag="pss")
                nc.tensor.matmul(psb[:, :], selb_t[:, :], recip_sb[:, fsl],
                                 start=True, stop=True)
                nc.scalar.activation(recipb[:, fsl], psb[:, :], Copy)
                # --- normalize into mask2 (slot e=1 <- row 2hp, e=0 <- 2hp+1)
                FE = HC * W                      # expT free size
                FM = (HC // 2) * W * 4 * 2       # mask2 free size
                for ij in range(4):
                    for e in range(2):
                        in0 = bass.AP(expT[:, :].tensor,
                                      32 * ij * FE + h0 * W + (1 - e) * W,
                                      [[FE, 25], [2 * W, 4], [1, W]])
                        in1 = bass.AP(recipb[:, :].tensor,
                                      32 * ij * FE + h0 * W + (1 - e) * W,
                                      [[FE, 25], [2 * W, 4], [1, W]])
                        outm = bass.AP(mask2[:, :, :, :, :].tensor,
                                       (h0 // 2) * 512 + ij * 2 + e,
                                       [[FM, 25], [512, 4], [8, W]])
                        nc.vector.tensor_mul(outm, in0, in1)

            # --- banded scatter + reassembly, pipelined over row-pairs ----
            def reasm_chunk(hc2):
                FM = (HC // 2) * W * 4 * 2
                for hp in range(hc2 * 4, hc2 * 4 + 4):
                    h = 2 * hp                   # even local row
                    src = bass.AP(mask2[:, :, :, :, :].tensor, hp * 512,
                                  [[FM, 25], [8, W], [1, 8]])
                    dst = bass.AP(Bd[:].tensor, hp * SH,
                                  [[512, 25], [PB + 8, W], [1, 8]])
                    nc.sync.dma_start(dst, src)
                    pt = pair_t[hp % 4]
                    # load band rows into slots s=1..5; s=0/6 stay zero.
                    nc.gpsimd.dma_start(
                        pt[:, 1:6, :],
                        Bd[hp * SH + 2 * PB:(hp + 1) * SH].rearrange(
                            "(p s f) -> p s f", p=66, s=5))
                    for ct in range(2):
                        psm = psr.tile([128, 2, 64, 2, 2], fp32, tag="psr")
                        for r in range(6):
                            # col (e,w,ij) needs slot s = r + e, elem
                            # w*8 + ij*2 + e  ->  e-step = 512+1 = 513
                            mov = bass.AP(pt.tensor, r * 512,
                                          [[7 * 512, 66], [513, 2],
                                           [8, W], [2, 4]])
                            nc.tensor.matmul(
                                psm[:, :, :, :, :],
                                xT[:, h + r, ct, :],
                                mov,
                                start=(r == 0), stop=(r == 5))
                        stage = stagep.tile([128, 4, 64, 2], bf16,
                                            tag="stage")
                        # e=1 -> stage rows 0:2 (out 4hp..), e=0 -> rows 2:4
                        for e in range(2):
                            nc.scalar.activation(
                                stage[:, 2 * (1 - e):2 * (1 - e) + 2, :, :],
                                psm[:, e, :, :, :].rearrange(
                                    "p w i j -> p i w j"),
                                Copy)
                        nc.sync.dma_start(
                            outp[ct * 128:(ct + 1) * 128,
                                 4 * hp:4 * hp + 4, :],
                            stage[:, :, :, :].rearrange(
                                "p r w j -> p r (w j)"))

            # one-chunk skew: band chains get a chunk of PE-time headroom
            mask_chunk(0)
            mask_chunk(1)
            reasm_chunk(0)
            mask_chunk(2)
            reasm_chunk(1)
            mask_chunk(3)
            reasm_chunk(2)
            reasm_chunk(3)
    return nc


# ---------------------------------------------------------------------------
_cache = {}


def _get_nc():
    if "nc" not in _cache:
        _install_legalizer()
        nc = bass.Bass()
        build_carafe(nc)
        _cache["nc"] = nc
    return _cache["nc"]


def _prep_inputs(x, Wc, bc, We, be):
    x = np.asarray(x, np.float32)
    N = x.shape[0]
    WcT = np.ascontiguousarray(
        np.transpose(np.asarray(Wc)[:, :, 0, 0], (1, 0)).reshape(2, 128, CC)
    ).astype(BF16)
    # Encoder channel layout: partition 32*(2i+j) + 5*dx + dy
    # orig channel = (5*dy+dx)*4 + 2*i+j ; unused partitions zero.
    We = np.asarray(We)
    be = np.asarray(be)
    Wep = np.zeros((128, CC, 3, 3), We.dtype)
    bep = np.zeros((128, 1), np.float32)
    sel = np.zeros((128, 4), BF16)
    selb = np.zeros((4, 128), BF16)
    for ij in range(4):
        for dx in range(5):
            for dy in range(5):
                p = 32 * ij + 5 * dx + dy
                o = (5 * dy + dx) * 4 + ij
                Wep[p] = We[o]
                bep[p, 0] = be[o]
                sel[p, ij] = 1
                selb[ij, p] = 1
    wet = np.ascontiguousarray(
        Wep.transpose(1, 2, 3, 0).reshape(CC, 9, 128)).astype(BF16)
    ident = np.eye(128, dtype=BF16)
    bcx = np.asarray(bc).astype(np.float32).reshape(CC, 1)

    xp = np.zeros((N, C, H + 4, W), BF16)
    xp[:, :, 2:H + 2] = x
    in_maps = []
    for core in range(NCORES):
        n, hh = core // 2, core % 2
        xs = np.ascontiguousarray(xp[n, :, hh * HC:hh * HC + RW, :])
        in_maps.append({
            "xs": xs, "wc": WcT, "we": wet, "bc": bcx, "be": bep,
            "sel": sel, "selb": selb, "ident": ident,
        })
    return in_maps


def kernel(x, Wc, bc, We, be):
    nc = _get_nc()
    in_maps = _prep_inputs(x, Wc, bc, We, be)
    res = bass_utils.run_bass_kernel_spmd(nc, in_maps, list(range(NCORES)))
    N = np.asarray(x).shape[0]
    full = np.empty((N, C, 2 * H, 2 * W), np.float32)
    for core in range(NCORES):
        n, hh = core // 2, core % 2
        full[n, :, 64 * hh:64 * hh + 64, :] = \
            res.results[core]["out"].astype(np.float32)
    return full


# revision 44
# speedup vs baseline: 3.3385x; 1.2086x over previous
"""CARAFE upsample (N=4, C=256, 64x64 -> 128x128, K=5, SF=2) on 8 NeuronCores.

Bass/Tile kernel, SPMD over 8 cores: core k handles batch n = k//2 and
row-half hh = k%2 (32 source rows, full 256 channels).

Per-core pipeline:
  1. load x row-window [256, 36, 64] fp32 (host zero-padded rows), cast bf16
  2. PE-transpose x -> xT [66 p (x col w, +2 zero rows), 36 r, 2 ct, 128 c]
  3. compressor 1x1 conv (PE): comp [64, 34 r, 66 wpad] bf16
  4. per 8-row chunk: encoder 3x3 conv (9 shifted matmuls, output channels
     at partitions 32*(2i+j) + 5*dx + dy) -> softmax over the 25 taps
     (exp on ACT, tap-sum + reciprocal + broadcast via tiny matmuls)
     -> mask2 [25 p (k'=5dx+dy), 32 h, 64 w, 4 ij] bf16
  5. banded scatter mask2 -> DRAM scratch Bd (diagonal strides legal on the
     flat DRAM side; 2 guard rows at the front absorb w+dx-2 < 0), then
     reload per h as B [66 p, 5 dy, 256 (w,ij)]
  6. reassembly per (h, ct): 5 PSUM-accumulating matmuls
       psum[c, (w,i,j)] = sum_dy xT_row(h+dy)^T @ B(h, dy)
  7. ACT evac reorders (w,i,j)->(i,w,j), DMA out rows 2h, 2h+1.

Contract: kernel(**inputs) -> full (4, 256, 128, 128) float32.
"""
import json
import numpy as np
import ml_dtypes

import concourse.bass as bass
import concourse.mybir as mybir
from concourse.tile import TileContext
from concourse import bass_utils

BF16 = ml_dtypes.bfloat16

H = 64
W = 64
C = 256
CC = 64
NCORES = 8
HC = 32           # rows per core
RW = 36           # loaded row window (HC + 2*2 halo)
PB = 5 * 512      # banded row elems per row-PAIR (5 dy blocks of (w,ij,e)=512)
SH = 68 * PB      # Bd elems per pair: 2 guard rows + 66 data rows


# ---------------------------------------------------------------------------
# BIR legalization: this walrus build accepts at most one sync-wait per
# instruction; hoist extras into standalone EventSemaphore instructions.
def _legalize_bir_json(bir: bytes) -> bytes:
    m = json.loads(bir)
    for fn in m.get("functions", []):
        for blk in fn.get("blocks", []):
            out = []
            for inst in blk.get("instructions", []):
                si = inst.get("sync_info") or {}
                waits = si.get("on_wait") or []
                if len(waits) > 1:
                    for k, wcond in enumerate(waits[:-1]):
                        out.append({
                            "debug": inst.get("debug", 0),
                            "engine": inst.get("engine"),
                            "ins": [],
                            "name": f"{inst.get('name', 'I')}_hw{k}",
                            "opcode": "EventSemaphore",
                            "outs": [],
                            "sync_info": {"on_update": [], "on_wait": [wcond]},
                        })
                    si = dict(si)
                    si["on_wait"] = [waits[-1]]
                    inst = dict(inst)
                    inst["sync_info"] = si
                out.append(inst)
            blk["instructions"] = out
    return json.dumps(m).encode()


_patched = False


def _install_legalizer():
    global _patched
    if _patched:
        return
    _patched = True
    orig = bass_utils.compile_bir_kernel

    def patched(bir_json, tmpdir, neff_name="file.neff"):
        if isinstance(bir_json, str):
            bir_json = bir_json.encode()
        return orig(_legalize_bir_json(bir_json), tmpdir, neff_name)

    bass_utils.compile_bir_kernel = patched
    try:
        from concourse import bass2jax
        bass2jax.compile_bir_kernel = patched
    except Exception:
        pass


# ---------------------------------------------------------------------------
def build_carafe(nc: bass.Bass):
    fp32 = mybir.dt.float32
    bf16 = mybir.dt.bfloat16
    Copy = mybir.ActivationFunctionType.Copy
    Ident = mybir.ActivationFunctionType.Identity
    Exp = mybir.ActivationFunctionType.Exp

    xs = nc.dram_tensor("xs", (C, RW, W), bf16, kind="ExternalInput")
    wc = nc.dram_tensor("wc", (2, 128, CC), bf16, kind="ExternalInput")
    we = nc.dram_tensor("we", (CC, 9, 128), bf16, kind="ExternalInput")
    bc_d = nc.dram_tensor("bc", (CC, 1), fp32, kind="ExternalInput")
    be_d = nc.dram_tensor("be", (128, 1), fp32, kind="ExternalInput")
    sel_d = nc.dram_tensor("sel", (128, 4), bf16, kind="ExternalInput")
    selb_d = nc.dram_tensor("selb", (4, 128), bf16, kind="ExternalInput")
    id_d = nc.dram_tensor("ident", (128, 128), bf16, kind="ExternalInput")
    outp = nc.dram_tensor("out", (C, 2 * HC, 2 * W), bf16, kind="ExternalOutput")
    Bd = nc.dram_tensor("Bd", (HC // 2 * SH,), bf16, kind="Internal")

    with nc.allow_low_precision(reason="bf16 pipeline, tol 2e-2"), \
         TileContext(nc) as tc:
        with (
            tc.tile_pool(name="const", bufs=1) as constp,
            tc.tile_pool(name="data", bufs=1) as datap,
            tc.tile_pool(name="stagep", bufs=6) as stagep,
            tc.tile_pool(name="pst", bufs=2, space="PSUM") as pst,
            tc.tile_pool(name="psc", bufs=2, space="PSUM") as psc,
            tc.tile_pool(name="pss", bufs=1, space="PSUM") as pss,
            tc.tile_pool(name="psr", bufs=3, space="PSUM") as psr,
        ):
            wc_t = constp.tile([128, 2, CC], bf16, tag="wc")
            we_t = constp.tile([CC, 9, 128], bf16, tag="we")
            bc_t = constp.tile([CC, 1], fp32, tag="bc")
            be_t = constp.tile([128, 1], fp32, tag="be")
            sel_t = constp.tile([128, 4], bf16, tag="sel")
            selb_t = constp.tile([4, 128], bf16, tag="selb")
            id_t = constp.tile([128, 128], bf16, tag="ident")
            zero_t = constp.tile([68, PB], bf16, tag="zero")

            nc.sync.dma_start(id_t[:, :], id_d[:, :])
            xb = datap.tile([128, 2, RW, W], bf16, tag="xb")
            xsr = xs.rearrange("(t p) r w -> p t r w", p=128)
            for ct in range(2):
                for r0 in range(0, RW, 12):
                    nc.sync.dma_start(xb[:, ct, r0:r0 + 12, :],
                                      xsr[:, ct, r0:r0 + 12, :])
            nc.sync.dma_start(wc_t[:, :, :], wc.rearrange("t p c -> p t c"))
            nc.sync.dma_start(we_t[:, :, :], we[:, :, :])
            nc.sync.dma_start(bc_t[:, :], bc_d[:, :])
            nc.sync.dma_start(be_t[:, :], be_d[:, :])
            nc.sync.dma_start(sel_t[:, :], sel_d[:, :])
            nc.sync.dma_start(selb_t[:, :], selb_d[:, :])
            nc.vector.memset(zero_t[:, :], 0.0)
            # zero-fill all banded scratch rows up front (no deps)
            for hp in range(HC // 2):
                nc.gpsimd.dma_start(
                    Bd[hp * SH:(hp + 1) * SH].rearrange("(p f) -> p f", p=68),
                    zero_t[:, 0:PB])

            # --- transpose x into xT [66 p = x col, (r, ct, c)] ------------
            xT = datap.tile([66, RW, 2, 128], bf16, tag="xT")
            for q4 in range(4):
                nc.sync.dma_start(
                    xT[64:66, 9 * q4:9 * q4 + 9, :, :].rearrange(
                        "p r t c -> p (r t c)"),
                    zero_t[0:2, 0:9 * 2 * 128])
            for ct in range(2):
                for r0 in range(0, RW, 4):
                    ps = pst.tile([64, 512], bf16, tag="pst")
                    for i in range(4):
                        nc.tensor.transpose(
                            ps[:, i * 128:(i + 1) * 128],
                            xb[:, ct, r0 + i, :],
                            id_t[:, :])
                    nc.vector.tensor_copy(
                        xT[0:64, r0:r0 + 4, ct, :],
                        ps[:, :].rearrange("p (i c) -> p i c", i=4))

            # --- compressor 1x1 conv --------------------------------------
            # comp rows rc (0..34) = xs rows rc+1;  w padded to 66 cols
            comp = datap.tile([CC, 34, 66], bf16, tag="comp")
            nc.vector.memset(comp[:, :, 0], 0.0)
            nc.vector.memset(comp[:, :, 65], 0.0)
            for r0 in range(1, 35, 8):
                nr = min(8, 35 - r0)
                ps = psc.tile([128, 512], fp32, tag="psc")
                for ct in range(2):
                    nc.tensor.matmul(
                        ps[0:CC, 0:nr * W],
                        wc_t[:, ct, :],
                        xb[:, ct, r0:r0 + nr, :],
                        start=(ct == 0), stop=(ct == 1))
                nc.scalar.activation(
                    comp[:, r0 - 1:r0 - 1 + nr, 1:65],
                    ps[0:CC, 0:nr * W].rearrange("p (r w) -> p r w", r=nr),
                    Ident, bias=bc_t[:, :], scale=1.0)

            # persistent row-pair band tiles [66 p, s=7 slots, (w,ij,e)=512];
            # slots 0 and 6 are permanent zeros (band edge padding).
            pair_t = [datap.tile([66, 7, 512], bf16, tag=f"pair{i}",
                                 name=f"pair{i}")
                      for i in range(4)]
            for i in range(4):
                nc.sync.dma_start(pair_t[i][:, 0, :], zero_t[0:66, 0:512])
                nc.sync.dma_start(pair_t[i][:, 6, :], zero_t[0:66, 0:512])

            expT = datap.tile([128, HC * W], bf16, tag="expT")
            recip_sb = datap.tile([4, HC * W], bf16, tag="recip")
            recipb = datap.tile([128, HC * W], bf16, tag="recipb")
            # mask2 element layout: (pair hp, w, ij, e); slot e=1 holds the
            # even row 2hp, e=0 the odd row 2hp+1 (so the reassembly moving
            # operand has all-positive strides).
            mask2 = datap.tile([25, HC // 2, W, 4, 2], bf16, tag="mask2")

            def mask_chunk(hc):
                h0 = hc * 8
                fsl = slice(h0 * W, (h0 + 8) * W)
                # --- encoder 3x3 conv (9 shifted matmuls) -----------------
                pse = psc.tile([128, 512], fp32, tag="psc")
                t = 0
                for ky in range(3):
                    for kx in range(3):
                        nc.tensor.matmul(
                            pse[:, :],
                            we_t[:, t, :],
                            comp[:, h0 + ky:h0 + ky + 8, kx:kx + 64],
                            start=(t == 0), stop=(t == 8))
                        t += 1
                nc.scalar.activation(expT[:, fsl], pse[:, :], Exp,
                                     bias=be_t[:, :], scale=1.0)
                # --- softmax denominator / reciprocal ---------------------
                psd = pss.tile([128, 512], fp32, tag="pss")
                nc.tensor.matmul(psd[0:4, :], sel_t[:, :], expT[:, fsl],
                                 start=True, stop=True)
                nc.vector.reciprocal(recip_sb[:, fsl], psd[0:4, :])
                psb = pss.tile([128, 512], fp32, tag="pss")
                nc.tensor.matmul(psb[:, :], selb_t[:, :], recip_sb[:, fsl],
                                 start=True, stop=True)
                nc.scalar.activation(recipb[:, fsl], psb[:, :], Copy)
                # --- normalize into mask2 (slot e=1 <- row 2hp, e=0 <- 2hp+1)
                FE = HC * W                      # expT free size
                FM = (HC // 2) * W * 4 * 2       # mask2 free size
                for ij in range(4):
                    for e in range(2):
                        in0 = bass.AP(expT[:, :].tensor,
                                      32 * ij * FE + h0 * W + (1 - e) * W,
                                      [[FE, 25], [2 * W, 4], [1, W]])
                        in1 = bass.AP(recipb[:, :].tensor,
                                      32 * ij * FE + h0 * W + (1 - e) * W,
                                      [[FE, 25], [2 * W, 4], [1, W]])
                        outm = bass.AP(mask2[:, :, :, :, :].tensor,
                                       (h0 // 2) * 512 + ij * 2 + e,
                                       [[FM, 25], [512, 4], [8, W]])
                        nc.vector.tensor_mul(outm, in0, in1)

            # --- banded scatter + reassembly, pipelined over row-pairs ----
            def reasm_chunk(hc2):
                FM = (HC // 2) * W * 4 * 2
                for hp in range(hc2 * 4, hc2 * 4 + 4):
                    h = 2 * hp                   # even local row
                    src = bass.AP(mask2[:, :, :, :, :].tensor, hp * 512,
                                  [[FM, 25], [8, W], [1, 8]])
                    dst = bass.AP(Bd[:].tensor, hp * SH,
                                  [[512, 25], [PB + 8, W], [1, 8]])
                    nc.sync.dma_start(dst, src)
                    pt = pair_t[hp % 4]
                    # load band rows into slots s=1..5; s=0/6 stay zero.
                    nc.gpsimd.dma_start(
                        pt[:, 1:6, :],
                        Bd[hp * SH + 2 * PB:(hp + 1) * SH].rearrange(
                            "(p s f) -> p s f", p=66, s=5))
                    for ct in range(2):
                        psm = psr.tile([128, 2, 64, 2, 2], fp32, tag="psr")
                        for r in range(6):
                            # col (e,w,ij) needs slot s = r + e, elem
                            # w*8 + ij*2 + e  ->  e-step = 512+1 = 513
                            mov = bass.AP(pt.tensor, r * 512,
                                          [[7 * 512, 66], [513, 2],
                                           [8, W], [2, 4]])
                            nc.tensor.matmul(
                                psm[:, :, :, :, :],
                                xT[:, h + r, ct, :],
                                mov,
                                start=(r == 0), stop=(r == 5))
                        stage = stagep.tile([128, 4, 64, 2], bf16,
                                            tag="stage")
                        # e=1 -> stage rows 0:2 (out 4hp..), e=0 -> rows 2:4
                        for e in range(2):
                            nc.scalar.activation(
                                stage[:, 2 * (1 - e):2 * (1 - e) + 2, :, :],
                                psm[:, e, :, :, :].rearrange(
                                    "p w i j -> p i w j"),
                                Copy)
                        nc.sync.dma_start(
                            outp[ct * 128:(ct + 1) * 128,
                                 4 * hp:4 * hp + 4, :],
                            stage[:, :, :, :].rearrange(
                                "p r w j -> p r (w j)"))

            # one-chunk skew: band chains get a chunk of PE-time headroom
            mask_chunk(0)
            mask_chunk(1)
            reasm_chunk(0)
            mask_chunk(2)
            reasm_chunk(1)
            mask_chunk(3)
            reasm_chunk(2)
            reasm_chunk(3)
    return nc


# ---------------------------------------------------------------------------
_cache = {}


def _get_nc():
    if "nc" not in _cache:
        _install_legalizer()
        nc = bass.Bass()
        build_carafe(nc)
        _cache["nc"] = nc
    return _cache["nc"]


def _prep_inputs(x, Wc, bc, We, be):
    x = np.asarray(x, np.float32)
    N = x.shape[0]
    WcT = np.ascontiguousarray(
        np.transpose(np.asarray(Wc)[:, :, 0, 0], (1, 0)).reshape(2, 128, CC)
    ).astype(BF16)
    # Encoder channel layout: partition 32*(2i+j) + 5*dx + dy
    # orig channel = (5*dy+dx)*4 + 2*i+j ; unused partitions zero.
    We = np.asarray(We)
    be = np.asarray(be)
    Wep = np.zeros((128, CC, 3, 3), We.dtype)
    bep = np.zeros((128, 1), np.float32)
    sel = np.zeros((128, 4), BF16)
    selb = np.zeros((4, 128), BF16)
    for ij in range(4):
        for dx in range(5):
            for dy in range(5):
                p = 32 * ij + 5 * dx + dy
                o = (5 * dy + dx) * 4 + ij
                Wep[p] = We[o]
                bep[p, 0] = be[o]
                sel[p, ij] = 1
                selb[ij, p] = 1
    wet = np.ascontiguousarray(
        Wep.transpose(1, 2, 3, 0).reshape(CC, 9, 128)).astype(BF16)
    ident = np.eye(128, dtype=BF16)
    bcx = np.asarray(bc).astype(np.float32).reshape(CC, 1)

    xp = np.zeros((N, C, H + 4, W), BF16)
    xp[:, :, 2:H + 2] = x
    in_maps = []
    for core in range(NCORES):
        n, hh = core // 2, core % 2
        xs = np.ascontiguousarray(xp[n, :, hh * HC:hh * HC + RW, :])
        in_maps.append({
            "xs": xs, "wc": WcT, "we": wet, "bc": bcx, "be": bep,
            "sel": sel, "selb": selb, "ident": ident,
        })
    return in_maps


def kernel(x, Wc, bc, We, be):
    nc = _get_nc()
    in_maps = _prep_inputs(x, Wc, bc, We, be)
    res = bass_utils.run_bass_kernel_spmd(nc, in_maps, list(range(NCORES)))
    N = np.asarray(x).shape[0]
    full = np.empty((N, C, 2 * H, 2 * W), np.float32)
    for core in range(NCORES):
        n, hh = core // 2, core % 2
        full[n, :, 64 * hh:64 * hh + 64, :] = \
            res.results[core]["out"].astype(np.float32)
    return full


# revision 45
# speedup vs baseline: 3.4512x; 1.0337x over previous
"""CARAFE upsample (N=4, C=256, 64x64 -> 128x128, K=5, SF=2) on 8 NeuronCores.

Bass/Tile kernel, SPMD over 8 cores: core k handles batch n = k//2 and
row-half hh = k%2 (32 source rows, full 256 channels).

Per-core pipeline:
  1. load x row-window [256, 36, 64] bf16 (host zero-padded rows + cast)
  2. PE-transpose x -> xT [66 p (x col w, +2 zero cols), 36 r, 2 ct, 128 c]
  3. compressor 1x1 conv (PE): comp [64, 34 r, 66 wpad] bf16
  4. per 8-row chunk: encoder 3x3 conv (9 shifted matmuls, output channels
     at partitions 32*(2i+j) + 5*dx + dy) -> softmax over the 25 taps
     (exp on ACT, tap-sum + DVE reciprocal + broadcast via tiny matmuls)
     -> mask2 [25 p (k'=5dx+dy), 16 hp, 64 w, 4 ij, 2 e] bf16, where
     slot e=1 holds even row 2hp and e=0 the odd row 2hp+1
  5. banded scatter mask2 -> DRAM scratch Bd in one 3-dim DMA per row-pair
     (diagonal strides are legal on the flat DRAM side; k'-order makes the
     25 taps one uniform-stride dim; 2 zero-filled guard rows at the front
     of each pair block absorb w+dx-2 < 0), then reload per pair into a
     7-slot tile [66 p, s, 512 (w,ij,e)] whose slots 0/6 stay zero
  6. reassembly per (pair, ct): 6 PSUM-accumulating matmuls, one per
     stationary row r; the moving operand's e-dim step of 513 pairs slot
     s = r+e with element offset e, so both pair rows share each matmul:
       psum[c, (e,w,i,j)] = sum_r xT_row(2hp+r)^T @ B_slots(r..r+1)
  7. two ACT evacs reorder (e,w,i,j)->(row,i,w,j) into one bf16 stage
     tile, one DMA stores output rows 4hp..4hp+4.

Contract: kernel(**inputs) -> full (4, 256, 128, 128) float32
(device output is bf16; host casts on gather).
"""
import json
import numpy as np
import ml_dtypes

import concourse.bass as bass
import concourse.mybir as mybir
from concourse.tile import TileContext
from concourse import bass_utils

BF16 = ml_dtypes.bfloat16

H = 64
W = 64
C = 256
CC = 64
NCORES = 8
HC = 32           # rows per core
RW = 36           # loaded row window (HC + 2*2 halo)
PB = 5 * 512      # banded row elems per row-PAIR (5 dy blocks of (w,ij,e)=512)
SH = 68 * PB      # Bd elems per pair: 2 guard rows + 66 data rows


# ---------------------------------------------------------------------------
# BIR legalization: this walrus build accepts at most one sync-wait per
# instruction; hoist extras into standalone EventSemaphore instructions.
def _legalize_bir_json(bir: bytes) -> bytes:
    m = json.loads(bir)
    for fn in m.get("functions", []):
        for blk in fn.get("blocks", []):
            out = []
            for inst in blk.get("instructions", []):
                si = inst.get("sync_info") or {}
                waits = si.get("on_wait") or []
                if len(waits) > 1:
                    for k, wcond in enumerate(waits[:-1]):
                        out.append({
                            "debug": inst.get("debug", 0),
                            "engine": inst.get("engine"),
                            "ins": [],
                            "name": f"{inst.get('name', 'I')}_hw{k}",
                            "opcode": "EventSemaphore",
                            "outs": [],
                            "sync_info": {"on_update": [], "on_wait": [wcond]},
                        })
                    si = dict(si)
                    si["on_wait"] = [waits[-1]]
                    inst = dict(inst)
                    inst["sync_info"] = si
                out.append(inst)
            blk["instructions"] = out
    return json.dumps(m).encode()


_patched = False


def _install_legalizer():
    global _patched
    if _patched:
        return
    _patched = True
    orig = bass_utils.compile_bir_kernel

    def patched(bir_json, tmpdir, neff_name="file.neff"):
        if isinstance(bir_json, str):
            bir_json = bir_json.encode()
        return orig(_legalize_bir_json(bir_json), tmpdir, neff_name)

    bass_utils.compile_bir_kernel = patched
    try:
        from concourse import bass2jax
        bass2jax.compile_bir_kernel = patched
    except Exception:
        pass


# ---------------------------------------------------------------------------
def build_carafe(nc: bass.Bass):
    fp32 = mybir.dt.float32
    bf16 = mybir.dt.bfloat16
    Copy = mybir.ActivationFunctionType.Copy
    Ident = mybir.ActivationFunctionType.Identity
    Exp = mybir.ActivationFunctionType.Exp

    xs = nc.dram_tensor("xs", (C, RW, W), bf16, kind="ExternalInput")
    wc = nc.dram_tensor("wc", (2, 128, CC), bf16, kind="ExternalInput")
    we = nc.dram_tensor("we", (CC, 9, 128), bf16, kind="ExternalInput")
    bc_d = nc.dram_tensor("bc", (CC, 1), fp32, kind="ExternalInput")
    be_d = nc.dram_tensor("be", (128, 1), fp32, kind="ExternalInput")
    sel_d = nc.dram_tensor("sel", (128, 4), bf16, kind="ExternalInput")
    selb_d = nc.dram_tensor("selb", (4, 128), bf16, kind="ExternalInput")
    id_d = nc.dram_tensor("ident", (128, 128), bf16, kind="ExternalInput")
    outp = nc.dram_tensor("out", (C, 2 * HC, 2 * W), bf16, kind="ExternalOutput")
    Bd = nc.dram_tensor("Bd", (HC // 2 * SH,), bf16, kind="Internal")

    with nc.allow_low_precision(reason="bf16 pipeline, tol 2e-2"), \
         TileContext(nc) as tc:
        with (
            tc.tile_pool(name="const", bufs=1) as constp,
            tc.tile_pool(name="data", bufs=1) as datap,
            tc.tile_pool(name="stagep", bufs=6) as stagep,
            tc.tile_pool(name="pst", bufs=2, space="PSUM") as pst,
            tc.tile_pool(name="psc", bufs=2, space="PSUM") as psc,
            tc.tile_pool(name="pss", bufs=1, space="PSUM") as pss,
            tc.tile_pool(name="psr", bufs=3, space="PSUM") as psr,
        ):
            wc_t = constp.tile([128, 2, CC], bf16, tag="wc")
            we_t = constp.tile([CC, 9, 128], bf16, tag="we")
            bc_t = constp.tile([CC, 1], fp32, tag="bc")
            be_t = constp.tile([128, 1], fp32, tag="be")
            sel_t = constp.tile([128, 4], bf16, tag="sel")
            selb_t = constp.tile([4, 128], bf16, tag="selb")
            id_t = constp.tile([128, 128], bf16, tag="ident")
            zero_t = constp.tile([68, PB], bf16, tag="zero")

            nc.sync.dma_start(id_t[:, :], id_d[:, :])
            xb = datap.tile([128, 2, RW, W], bf16, tag="xb")
            xsr = xs.rearrange("(t p) r w -> p t r w", p=128)
            for ct in range(2):
                for r0 in range(0, RW, 12):
                    nc.sync.dma_start(xb[:, ct, r0:r0 + 12, :],
                                      xsr[:, ct, r0:r0 + 12, :])
            nc.sync.dma_start(wc_t[:, :, :], wc.rearrange("t p c -> p t c"))
            nc.sync.dma_start(we_t[:, :, :], we[:, :, :])
            nc.sync.dma_start(bc_t[:, :], bc_d[:, :])
            nc.sync.dma_start(be_t[:, :], be_d[:, :])
            nc.sync.dma_start(sel_t[:, :], sel_d[:, :])
            nc.sync.dma_start(selb_t[:, :], selb_d[:, :])
            nc.vector.memset(zero_t[:, :], 0.0)
            # zero-fill all banded scratch rows up front (no deps)
            for hp in range(HC // 2):
                nc.gpsimd.dma_start(
                    Bd[hp * SH:(hp + 1) * SH].rearrange("(p f) -> p f", p=68),
                    zero_t[:, 0:PB])

            # --- transpose x into xT [66 p = x col, (r, ct, c)] ------------
            xT = datap.tile([66, RW, 2, 128], bf16, tag="xT")
            for q4 in range(4):
                nc.sync.dma_start(
                    xT[64:66, 9 * q4:9 * q4 + 9, :, :].rearrange(
                        "p r t c -> p (r t c)"),
                    zero_t[0:2, 0:9 * 2 * 128])
            for ct in range(2):
                for r0 in range(0, RW, 4):
                    ps = pst.tile([64, 512], bf16, tag="pst")
                    for i in range(4):
                        nc.tensor.transpose(
                            ps[:, i * 128:(i + 1) * 128],
                            xb[:, ct, r0 + i, :],
                            id_t[:, :])
                    nc.vector.tensor_copy(
                        xT[0:64, r0:r0 + 4, ct, :],
                        ps[:, :].rearrange("p (i c) -> p i c", i=4))

            # --- compressor 1x1 conv --------------------------------------
            # comp rows rc (0..34) = xs rows rc+1;  w padded to 66 cols
            comp = datap.tile([CC, 34, 66], bf16, tag="comp")
            nc.vector.memset(comp[:, :, 0], 0.0)
            nc.vector.memset(comp[:, :, 65], 0.0)
            for r0 in range(1, 35, 8):
                nr = min(8, 35 - r0)
                ps = psc.tile([128, 512], fp32, tag="psc")
                for ct in range(2):
                    nc.tensor.matmul(
                        ps[0:CC, 0:nr * W],
                        wc_t[:, ct, :],
                        xb[:, ct, r0:r0 + nr, :],
                        start=(ct == 0), stop=(ct == 1))
                nc.scalar.activation(
                    comp[:, r0 - 1:r0 - 1 + nr, 1:65],
                    ps[0:CC, 0:nr * W].rearrange("p (r w) -> p r w", r=nr),
                    Ident, bias=bc_t[:, :], scale=1.0)

            # persistent row-pair band tiles [66 p, s=7 slots, (w,ij,e)=512];
            # slots 0 and 6 are permanent zeros (band edge padding).
            pair_t = [datap.tile([66, 7, 512], bf16, tag=f"pair{i}",
                                 name=f"pair{i}")
                      for i in range(4)]
            for i in range(4):
                nc.sync.dma_start(pair_t[i][:, 0, :], zero_t[0:66, 0:512])
                nc.sync.dma_start(pair_t[i][:, 6, :], zero_t[0:66, 0:512])

            expT = datap.tile([128, HC * W], bf16, tag="expT")
            recip_sb = datap.tile([4, HC * W], bf16, tag="recip")
            recipb = datap.tile([128, HC * W], bf16, tag="recipb")
            # mask2 element layout: (pair hp, w, ij, e); slot e=1 holds the
            # even row 2hp, e=0 the odd row 2hp+1 (so the reassembly moving
            # operand has all-positive strides).
            mask2 = datap.tile([25, HC // 2, W, 4, 2], bf16, tag="mask2")

            def mask_chunk(hc):
                h0 = hc * 8
                fsl = slice(h0 * W, (h0 + 8) * W)
                # --- encoder 3x3 conv (9 shifted matmuls) -----------------
                pse = psc.tile([128, 512], fp32, tag="psc")
                t = 0
                for ky in range(3):
                    for kx in range(3):
                        nc.tensor.matmul(
                            pse[:, :],
                            we_t[:, t, :],
                            comp[:, h0 + ky:h0 + ky + 8, kx:kx + 64],
                            start=(t == 0), stop=(t == 8))
                        t += 1
                nc.scalar.activation(expT[:, fsl], pse[:, :], Exp,
                                     bias=be_t[:, :], scale=1.0)
                # --- softmax denominator / reciprocal ---------------------
                psd = pss.tile([128, 512], fp32, tag="pss")
                nc.tensor.matmul(psd[0:4, :], sel_t[:, :], expT[:, fsl],
                                 start=True, stop=True)
                nc.vector.reciprocal(recip_sb[:, fsl], psd[0:4, :])
                psb = pss.tile([128, 512], fp32, tag="pss")
                nc.tensor.matmul(psb[:, :], selb_t[:, :], recip_sb[:, fsl],
                                 start=True, stop=True)
                nc.scalar.activation(recipb[:, fsl], psb[:, :], Copy)
                # --- normalize into mask2 (slot e=1 <- row 2hp, e=0 <- 2hp+1)
                FE = HC * W                      # expT free size
                FM = (HC // 2) * W * 4 * 2       # mask2 free size
                for ij in range(4):
                    for e in range(2):
                        in0 = bass.AP(expT[:, :].tensor,
                                      32 * ij * FE + h0 * W + (1 - e) * W,
                                      [[FE, 25], [2 * W, 4], [1, W]])
                        in1 = bass.AP(recipb[:, :].tensor,
                                      32 * ij * FE + h0 * W + (1 - e) * W,
                                      [[FE, 25], [2 * W, 4], [1, W]])
                        outm = bass.AP(mask2[:, :, :, :, :].tensor,
                                       (h0 // 2) * 512 + ij * 2 + e,
                                       [[FM, 25], [512, 4], [8, W]])
                        nc.vector.tensor_mul(outm, in0, in1)

            # --- banded scatter + reassembly, pipelined over row-pairs ----
            def reasm_chunk(hc2):
                FM = (HC // 2) * W * 4 * 2
                for hp in range(hc2 * 4, hc2 * 4 + 4):
                    h = 2 * hp                   # even local row
                    src = bass.AP(mask2[:, :, :, :, :].tensor, hp * 512,
                                  [[FM, 25], [8, W], [1, 8]])
                    dst = bass.AP(Bd[:].tensor, hp * SH,
                                  [[512, 25], [PB + 8, W], [1, 8]])
                    nc.sync.dma_start(dst, src)
                    pt = pair_t[hp % 4]
                    # load band rows into slots s=1..5; s=0/6 stay zero.
                    nc.gpsimd.dma_start(
                        pt[:, 1:6, :],
                        Bd[hp * SH + 2 * PB:(hp + 1) * SH].rearrange(
                            "(p s f) -> p s f", p=66, s=5))
                    for ct in range(2):
                        psm = psr.tile([128, 2, 64, 2, 2], fp32, tag="psr")
                        for r in range(6):
                            # col (e,w,ij) needs slot s = r + e, elem
                            # w*8 + ij*2 + e  ->  e-step = 512+1 = 513
                            mov = bass.AP(pt.tensor, r * 512,
                                          [[7 * 512, 66], [513, 2],
                                           [8, W], [2, 4]])
                            nc.tensor.matmul(
                                psm[:, :, :, :, :],
                                xT[:, h + r, ct, :],
                                mov,
                                start=(r == 0), stop=(r == 5))
                        stage = stagep.tile([128, 4, 64, 2], bf16,
                                            tag="stage")
                        # e=1 -> stage rows 0:2 (out 4hp..), e=0 -> rows 2:4
                        for e in range(2):
                            nc.scalar.activation(
                                stage[:, 2 * (1 - e):2 * (1 - e) + 2, :, :],
                                psm[:, e, :, :, :].rearrange(
                                    "p w i j -> p i w j"),
                                Copy)
                        nc.sync.dma_start(
                            outp[ct * 128:(ct + 1) * 128,
                                 4 * hp:4 * hp + 4, :],
                            stage[:, :, :, :].rearrange(
                                "p r w j -> p r (w j)"))

            # one-chunk skew: band chains get a chunk of PE-time headroom
            mask_chunk(0)
            mask_chunk(1)
            reasm_chunk(0)
            mask_chunk(2)
            reasm_chunk(1)
            mask_chunk(3)
            reasm_chunk(2)
            reasm_chunk(3)
    return nc


# ---------------------------------------------------------------------------
_cache = {}


def _get_nc():
    if "nc" not in _cache:
        _install_legalizer()
        nc = bass.Bass()
        build_carafe(nc)
        _cache["nc"] = nc
    return _cache["nc"]


def _prep_inputs(x, Wc, bc, We, be):
    x = np.asarray(x, np.float32)
    N = x.shape[0]
    WcT = np.ascontiguousarray(
        np.transpose(np.asarray(Wc)[:, :, 0, 0], (1, 0)).reshape(2, 128, CC)
    ).astype(BF16)
    # Encoder channel layout: partition 32*(2i+j) + 5*dx + dy
    # orig channel = (5*dy+dx)*4 + 2*i+j ; unused partitions zero.
    We = np.asarray(We)
    be = np.asarray(be)
    Wep = np.zeros((128, CC, 3, 3), We.dtype)
    bep = np.zeros((128, 1), np.float32)
    sel = np.zeros((128, 4), BF16)
    selb = np.zeros((4, 128), BF16)
    for ij in range(4):
        for dx in range(5):
            for dy in range(5):
                p = 32 * ij + 5 * dx + dy
                o = (5 * dy + dx) * 4 + ij
                Wep[p] = We[o]
                bep[p, 0] = be[o]
                sel[p, ij] = 1
                selb[ij, p] = 1
    wet = np.ascontiguousarray(
        Wep.transpose(1, 2, 3, 0).reshape(CC, 9, 128)).astype(BF16)
    ident = np.eye(128, dtype=BF16)
    bcx = np.asarray(bc).astype(np.float32).reshape(CC, 1)

    xp = np.zeros((N, C, H + 4, W), BF16)
    xp[:, :, 2:H + 2] = x
    in_maps = []
    for core in range(NCORES):
        n, hh = core // 2, core % 2
        xs = np.ascontiguousarray(xp[n, :, hh * HC:hh * HC + RW, :])
        in_maps.append({
            "xs": xs, "wc": WcT, "we": wet, "bc": bcx, "be": bep,
            "sel": sel, "selb": selb, "ident": ident,
        })
    return in_maps


def kernel(x, Wc, bc, We, be):
    nc = _get_nc()
    in_maps = _prep_inputs(x, Wc, bc, We, be)
    res = bass_utils.run_bass_kernel_spmd(nc, in_maps, list(range(NCORES)))
    N = np.asarray(x).shape[0]
    full = np.empty((N, C, 2 * H, 2 * W), np.float32)
    for core in range(NCORES):
        n, hh = core // 2, core % 2
        full[n, :, 64 * hh:64 * hh + 64, :] = \
            res.results[core]["out"].astype(np.float32)
    return full


# revision 46
# speedup vs baseline: 3.8728x; 1.1222x over previous
"""CARAFE upsample (N=4, C=256, 64x64 -> 128x128, K=5, SF=2) on 8 NeuronCores.

Bass/Tile kernel, SPMD over 8 cores: core k handles batch n = k//2 and
row-half hh = k%2 (32 source rows, full 256 channels).

Per-core pipeline:
  1. load x row-window [256, 36, 64] bf16 (host zero-padded rows + cast)
  2. PE-transpose x -> xT [66 p (x col w, +2 zero cols), 36 r, 2 ct, 128 c]
  3. compressor 1x1 conv (PE): comp [64, 34 r, 66 wpad] bf16
  4. per 8-row chunk: encoder 3x3 conv (9 shifted matmuls, output channels
     at partitions 32*(2i+j) + 5*dx + dy) -> softmax over the 25 taps
     (exp on ACT, tap-sum + DVE reciprocal + broadcast via tiny matmuls)
     -> mask2 [25 p (k'=5dx+dy), 16 hp, 64 w, 4 ij, 2 e] bf16, where
     slot e=1 holds even row 2hp and e=0 the odd row 2hp+1
  5. banded scatter mask2 -> DRAM scratch Bd in one 3-dim DMA per row-pair
     (diagonal strides are legal on the flat DRAM side; k'-order makes the
     25 taps one uniform-stride dim; 2 zero-filled guard rows at the front
     of each pair block absorb w+dx-2 < 0), then reload per pair into a
     7-slot tile [66 p, s, 512 (w,ij,e)] whose slots 0/6 stay zero
  6. reassembly per (pair, ct): 6 PSUM-accumulating matmuls, one per
     stationary row r; the moving operand's e-dim step of 513 pairs slot
     s = r+e with element offset e, so both pair rows share each matmul:
       psum[c, (e,w,i,j)] = sum_r xT_row(2hp+r)^T @ B_slots(r..r+1)
  7. two ACT evacs reorder (e,w,i,j)->(row,i,w,j) into one bf16 stage
     tile, one DMA stores output rows 4hp..4hp+4.

Contract: kernel(**inputs) -> full (4, 256, 128, 128) float32
(device output is bf16; host casts on gather).
"""
import json
import numpy as np
import ml_dtypes

import concourse.bass as bass
import concourse.mybir as mybir
from concourse.tile import TileContext
from concourse import bass_utils

BF16 = ml_dtypes.bfloat16

H = 64
W = 64
C = 256
CC = 64
NCORES = 8
HC = 32           # rows per core
RW = 36           # loaded row window (HC + 2*2 halo)
PB = 5 * 512      # banded row elems per row-PAIR (5 dy blocks of (w,ij,e)=512)
SH = 68 * PB      # Bd elems per pair: 2 guard rows + 66 data rows


# ---------------------------------------------------------------------------
# BIR legalization: this walrus build accepts at most one sync-wait per
# instruction; hoist extras into standalone EventSemaphore instructions.
def _legalize_bir_json(bir: bytes) -> bytes:
    m = json.loads(bir)
    for fn in m.get("functions", []):
        for blk in fn.get("blocks", []):
            out = []
            for inst in blk.get("instructions", []):
                si = inst.get("sync_info") or {}
                waits = si.get("on_wait") or []
                if len(waits) > 1:
                    for k, wcond in enumerate(waits[:-1]):
                        out.append({
                            "debug": inst.get("debug", 0),
                            "engine": inst.get("engine"),
                            "ins": [],
                            "name": f"{inst.get('name', 'I')}_hw{k}",
                            "opcode": "EventSemaphore",
                            "outs": [],
                            "sync_info": {"on_update": [], "on_wait": [wcond]},
                        })
                    si = dict(si)
                    si["on_wait"] = [waits[-1]]
                    inst = dict(inst)
                    inst["sync_info"] = si
                out.append(inst)
            blk["instructions"] = out
    return json.dumps(m).encode()


_patched = False


def _install_legalizer():
    global _patched
    if _patched:
        return
    _patched = True
    orig = bass_utils.compile_bir_kernel

    def patched(bir_json, tmpdir, neff_name="file.neff"):
        if isinstance(bir_json, str):
            bir_json = bir_json.encode()
        return orig(_legalize_bir_json(bir_json), tmpdir, neff_name)

    bass_utils.compile_bir_kernel = patched
    try:
        from concourse import bass2jax
        bass2jax.compile_bir_kernel = patched
    except Exception:
        pass


# ---------------------------------------------------------------------------
def build_carafe(nc: bass.Bass):
    fp32 = mybir.dt.float32
    bf16 = mybir.dt.bfloat16
    Copy = mybir.ActivationFunctionType.Copy
    Ident = mybir.ActivationFunctionType.Identity
    Exp = mybir.ActivationFunctionType.Exp

    xs = nc.dram_tensor("xs", (C, RW, W), bf16, kind="ExternalInput")
    wc = nc.dram_tensor("wc", (2, 128, CC), bf16, kind="ExternalInput")
    we = nc.dram_tensor("we", (CC, 9, 128), bf16, kind="ExternalInput")
    bc_d = nc.dram_tensor("bc", (CC, 1), fp32, kind="ExternalInput")
    be_d = nc.dram_tensor("be", (128, 1), fp32, kind="ExternalInput")
    sel_d = nc.dram_tensor("sel", (128, 4), bf16, kind="ExternalInput")
    selb_d = nc.dram_tensor("selb", (4, 128), bf16, kind="ExternalInput")
    id_d = nc.dram_tensor("ident", (128, 128), bf16, kind="ExternalInput")
    outp = nc.dram_tensor("out", (C, 2 * HC, 2 * W), bf16, kind="ExternalOutput")
    Bd = nc.dram_tensor("Bd", (HC // 2 * SH,), bf16, kind="Internal")

    with nc.allow_low_precision(reason="bf16 pipeline, tol 2e-2"), \
         TileContext(nc) as tc:
        with (
            tc.tile_pool(name="const", bufs=1) as constp,
            tc.tile_pool(name="data", bufs=1) as datap,
            tc.tile_pool(name="stagep", bufs=6) as stagep,
            tc.tile_pool(name="pst", bufs=2, space="PSUM") as pst,
            tc.tile_pool(name="psc", bufs=2, space="PSUM") as psc,
            tc.tile_pool(name="pss", bufs=1, space="PSUM") as pss,
            tc.tile_pool(name="psr", bufs=3, space="PSUM") as psr,
        ):
            wc_t = constp.tile([128, 2, CC], bf16, tag="wc")
            we_t = constp.tile([CC, 9, 128], bf16, tag="we")
            bc_t = constp.tile([CC, 1], fp32, tag="bc")
            be_t = constp.tile([128, 1], fp32, tag="be")
            sel_t = constp.tile([128, 4], bf16, tag="sel")
            selb_t = constp.tile([4, 128], bf16, tag="selb")
            id_t = constp.tile([128, 128], bf16, tag="ident")
            zero_t = constp.tile([68, PB], bf16, tag="zero")

            nc.sync.dma_start(id_t[:, :], id_d[:, :])
            xb = datap.tile([128, 2, RW, W], bf16, tag="xb")
            xsr = xs.rearrange("(t p) r w -> p t r w", p=128)
            for ct in range(2):
                for r0 in range(0, RW, 12):
                    nc.sync.dma_start(xb[:, ct, r0:r0 + 12, :],
                                      xsr[:, ct, r0:r0 + 12, :])
            nc.sync.dma_start(wc_t[:, :, :], wc.rearrange("t p c -> p t c"))
            nc.sync.dma_start(we_t[:, :, :], we[:, :, :])
            nc.sync.dma_start(bc_t[:, :], bc_d[:, :])
            nc.sync.dma_start(be_t[:, :], be_d[:, :])
            nc.sync.dma_start(sel_t[:, :], sel_d[:, :])
            nc.sync.dma_start(selb_t[:, :], selb_d[:, :])
            nc.vector.memset(zero_t[:, :], 0.0)
            # zero-fill all banded scratch rows up front (no deps)
            for hp in range(HC // 2):
                nc.gpsimd.dma_start(
                    Bd[hp * SH:(hp + 1) * SH].rearrange("(p f) -> p f", p=68),
                    zero_t[:, 0:PB])

            # --- transpose x into xT [66 p = x col, (r, ct, c)] ------------
            xT = datap.tile([66, RW, 2, 128], bf16, tag="xT")
            for q4 in range(4):
                nc.sync.dma_start(
                    xT[64:66, 9 * q4:9 * q4 + 9, :, :].rearrange(
                        "p r t c -> p (r t c)"),
                    zero_t[0:2, 0:9 * 2 * 128])
            for ct in range(2):
                for r0 in range(0, RW, 6):
                    ps = pst.tile([64, 6 * 128], bf16, tag="pst")
                    for i in range(6):
                        nc.tensor.transpose(
                            ps[:, i * 128:(i + 1) * 128],
                            xb[:, ct, r0 + i, :],
                            id_t[:, :])
                    nc.vector.tensor_copy(
                        xT[0:64, r0:r0 + 6, ct, :],
                        ps[:, :].rearrange("p (i c) -> p i c", i=6))

            # --- compressor 1x1 conv --------------------------------------
            # comp rows rc (0..34) = xs rows rc+1;  w padded to 66 cols
            comp = datap.tile([CC, 34, 66], bf16, tag="comp")
            nc.vector.memset(comp[:, :, 0], 0.0)
            nc.vector.memset(comp[:, :, 65], 0.0)
            for r0 in range(1, 35, 8):
                nr = min(8, 35 - r0)
                ps = psc.tile([128, 512], fp32, tag="psc")
                for ct in range(2):
                    nc.tensor.matmul(
                        ps[0:CC, 0:nr * W],
                        wc_t[:, ct, :],
                        xb[:, ct, r0:r0 + nr, :],
                        start=(ct == 0), stop=(ct == 1))
                nc.scalar.activation(
                    comp[:, r0 - 1:r0 - 1 + nr, 1:65],
                    ps[0:CC, 0:nr * W].rearrange("p (r w) -> p r w", r=nr),
                    Ident, bias=bc_t[:, :], scale=1.0)

            # persistent row-pair band tiles [66 p, s=7 slots, (w,ij,e)=512];
            # slots 0 and 6 are permanent zeros (band edge padding).
            pair_t = [datap.tile([66, 7, 512], bf16, tag=f"pair{i}",
                                 name=f"pair{i}")
                      for i in range(4)]
            for i in range(4):
                nc.sync.dma_start(pair_t[i][:, 0, :], zero_t[0:66, 0:512])
                nc.sync.dma_start(pair_t[i][:, 6, :], zero_t[0:66, 0:512])

            expT = datap.tile([128, HC * W], bf16, tag="expT")
            recip_sb = datap.tile([4, HC * W], bf16, tag="recip")
            recipb = datap.tile([128, HC * W], bf16, tag="recipb")
            # mask2 element layout: (pair hp, w, ij, e); slot e=1 holds the
            # even row 2hp, e=0 the odd row 2hp+1 (so the reassembly moving
            # operand has all-positive strides).
            mask2 = datap.tile([25, HC // 2, W, 4, 2], bf16, tag="mask2")

            def mask_chunk(hc):
                h0 = hc * 8
                fsl = slice(h0 * W, (h0 + 8) * W)
                # --- encoder 3x3 conv (9 shifted matmuls) -----------------
                pse = psc.tile([128, 512], fp32, tag="psc")
                t = 0
                for ky in range(3):
                    for kx in range(3):
                        nc.tensor.matmul(
                            pse[:, :],
                            we_t[:, t, :],
                            comp[:, h0 + ky:h0 + ky + 8, kx:kx + 64],
                            start=(t == 0), stop=(t == 8))
                        t += 1
                nc.scalar.activation(expT[:, fsl], pse[:, :], Exp,
                                     bias=be_t[:, :], scale=1.0)
                # --- softmax denominator / reciprocal ---------------------
                psd = pss.tile([128, 512], fp32, tag="pss")
                nc.tensor.matmul(psd[0:4, :], sel_t[:, :], expT[:, fsl],
                                 start=True, stop=True)
                nc.vector.reciprocal(recip_sb[:, fsl], psd[0:4, :])
                psb = pss.tile([128, 512], fp32, tag="pss")
                nc.tensor.matmul(psb[:, :], selb_t[:, :], recip_sb[:, fsl],
                                 start=True, stop=True)
                nc.scalar.activation(recipb[:, fsl], psb[:, :], Copy)
                # --- normalize into mask2 (slot e=1 <- row 2hp, e=0 <- 2hp+1)
                FE = HC * W                      # expT free size
                FM = (HC // 2) * W * 4 * 2       # mask2 free size
                for ij in range(4):
                    for e in range(2):
                        in0 = bass.AP(expT[:, :].tensor,
                                      32 * ij * FE + h0 * W + (1 - e) * W,
                                      [[FE, 25], [2 * W, 4], [1, W]])
                        in1 = bass.AP(recipb[:, :].tensor,
                                      32 * ij * FE + h0 * W + (1 - e) * W,
                                      [[FE, 25], [2 * W, 4], [1, W]])
                        outm = bass.AP(mask2[:, :, :, :, :].tensor,
                                       (h0 // 2) * 512 + ij * 2 + e,
                                       [[FM, 25], [512, 4], [8, W]])
                        nc.vector.tensor_mul(outm, in0, in1)

            # --- banded scatter + reassembly, pipelined over row-pairs ----
            def reasm_chunk(hc2):
                FM = (HC // 2) * W * 4 * 2
                for hp in range(hc2 * 4, hc2 * 4 + 4):
                    h = 2 * hp                   # even local row
                    src = bass.AP(mask2[:, :, :, :, :].tensor, hp * 512,
                                  [[FM, 25], [8, W], [1, 8]])
                    dst = bass.AP(Bd[:].tensor, hp * SH,
                                  [[512, 25], [PB + 8, W], [1, 8]])
                    nc.sync.dma_start(dst, src)
                    pt = pair_t[hp % 4]
                    # load band rows into slots s=1..5; s=0/6 stay zero.
                    nc.gpsimd.dma_start(
                        pt[:, 1:6, :],
                        Bd[hp * SH + 2 * PB:(hp + 1) * SH].rearrange(
                            "(p s f) -> p s f", p=66, s=5))
                    for ct in range(2):
                        psm = psr.tile([128, 2, 64, 2, 2], fp32, tag="psr")
                        for r in range(6):
                            # col (e,w,ij) needs slot s = r + e, elem
                            # w*8 + ij*2 + e  ->  e-step = 512+1 = 513
                            mov = bass.AP(pt.tensor, r * 512,
                                          [[7 * 512, 66], [513, 2],
                                           [8, W], [2, 4]])
                            nc.tensor.matmul(
                                psm[:, :, :, :, :],
                                xT[:, h + r, ct, :],
                                mov,
                                start=(r == 0), stop=(r == 5))
                        stage = stagep.tile([128, 4, 64, 2], bf16,
                                            tag="stage")
                        # e=1 -> stage rows 0:2 (out 4hp..), e=0 -> rows 2:4
                        for e in range(2):
                            nc.scalar.activation(
                                stage[:, 2 * (1 - e):2 * (1 - e) + 2, :, :],
                                psm[:, e, :, :, :].rearrange(
                                    "p w i j -> p i w j"),
                                Copy)
                        nc.sync.dma_start(
                            outp[ct * 128:(ct + 1) * 128,
                                 4 * hp:4 * hp + 4, :],
                            stage[:, :, :, :].rearrange(
                                "p r w j -> p r (w j)"))

            # one-chunk skew: band chains get a chunk of PE-time headroom
            mask_chunk(0)
            mask_chunk(1)
            reasm_chunk(0)
            mask_chunk(2)
            reasm_chunk(1)
            mask_chunk(3)
            reasm_chunk(2)
            reasm_chunk(3)
    return nc


# ---------------------------------------------------------------------------
_cache = {}


def _get_nc():
    if "nc" not in _cache:
        _install_legalizer()
        nc = bass.Bass()
        build_carafe(nc)
        _cache["nc"] = nc
    return _cache["nc"]


def _prep_inputs(x, Wc, bc, We, be):
    x = np.asarray(x, np.float32)
    N = x.shape[0]
    WcT = np.ascontiguousarray(
        np.transpose(np.asarray(Wc)[:, :, 0, 0], (1, 0)).reshape(2, 128, CC)
    ).astype(BF16)
    # Encoder channel layout: partition 32*(2i+j) + 5*dx + dy
    # orig channel = (5*dy+dx)*4 + 2*i+j ; unused partitions zero.
    We = np.asarray(We)
    be = np.asarray(be)
    Wep = np.zeros((128, CC, 3, 3), We.dtype)
    bep = np.zeros((128, 1), np.float32)
    sel = np.zeros((128, 4), BF16)
    selb = np.zeros((4, 128), BF16)
    for ij in range(4):
        for dx in range(5):
            for dy in range(5):
                p = 32 * ij + 5 * dx + dy
                o = (5 * dy + dx) * 4 + ij
                Wep[p] = We[o]
                bep[p, 0] = be[o]
                sel[p, ij] = 1
                selb[ij, p] = 1
    wet = np.ascontiguousarray(
        Wep.transpose(1, 2, 3, 0).reshape(CC, 9, 128)).astype(BF16)
    ident = np.eye(128, dtype=BF16)
    bcx = np.asarray(bc).astype(np.float32).reshape(CC, 1)

    xp = np.zeros((N, C, H + 4, W), BF16)
    xp[:, :, 2:H + 2] = x
    in_maps = []
    for core in range(NCORES):
        n, hh = core // 2, core % 2
        xs = np.ascontiguousarray(xp[n, :, hh * HC:hh * HC + RW, :])
        in_maps.append({
            "xs": xs, "wc": WcT, "we": wet, "bc": bcx, "be": bep,
            "sel": sel, "selb": selb, "ident": ident,
        })
    return in_maps


def kernel(x, Wc, bc, We, be):
    nc = _get_nc()
    in_maps = _prep_inputs(x, Wc, bc, We, be)
    res = bass_utils.run_bass_kernel_spmd(nc, in_maps, list(range(NCORES)))
    N = np.asarray(x).shape[0]
    full = np.empty((N, C, 2 * H, 2 * W), np.float32)
    for core in range(NCORES):
        n, hh = core // 2, core % 2
        full[n, :, 64 * hh:64 * hh + 64, :] = \
            res.results[core]["out"].astype(np.float32)
    return full
